# revision 83
# baseline (speedup 1.0000x reference)
"""Multi-head causal attention (B=2, T=2048, D=1024, H=16) on 8 TRN2 cores.

Sharding: core c handles batch b=c//4 and head group g=c%4 (4 heads each).

Device kernel per core:
  Projections: split-precision fp8 DoubleRow — host supplies X and 32*W as
    (hi, lo) e4m3 pairs; Q' = Xh@Wh + Xl@Wh + Xh@Wl = 32*(X@W) to ~fp16
    accuracy at 0.5 cyc/row (3 DR terms vs 8 fp16 matmuls: 1.33x faster
    and exact enough: W*32 keeps the lo residual above fp8's subnormal
    floor). Q/K stored fp16 (32x scaled), V fp16 (32x) with a ones-column
    per head (V_aug).
  S (fp16): per (q-tile 512, head-pair, key-block 128): ST[k,q] packed
    2 heads on the PE rows; S' = 1024*S so exp scale is 1/8192.
  PT = exp fp16; only the triangular diag subtile is masked (DVE).
  PV: PT[k, q-subtile 128] stationary, V_aug[k, 65] moving, C accumulates
    in [q, 4, 65] PSUM (col 64 = softmax denominator). One psum
    group per bank: only the first matmul starts, only the last stops.
  Epilogue: per-partition reciprocal + broadcast multiply (DVE),
    PE-transpose C_norm back to [c, q], out-proj fp16 against Wo/32.
Host: out[b] = sum over the 4 head-group cores of OT^T, + bo.

Scheduling: per q-tile, the 2*nkb S/exp stream paces the Activation
engine (exp floor ~58us); all other PE work — the previous q-tile's tail
chunks (wave-B PV, epilogues, transposes, out-proj), inline wave-A PV,
and next-tile projection fillers — is metered between s_exps with
dependency-aware priority and adaptive budgets. The last q-tile runs
wave-B inline since there is no later tile to hide it in.
"""

import numpy as np

B, T, D, H, HD = 2, 2048, 1024, 16, 64
NCORES = 8
HPC = 4          # heads per core
GC = HPC * HD    # 256 columns per head group
NQ = 512         # q-tile width
KB = 128         # key block

_cache = {}


def _build(t_len):
    from concourse import bacc
    import concourse.tile as tile
    import concourse.mybir as mybir
    from concourse.masks import make_identity

    dt = mybir.dt
    f32, f16, f8 = dt.float32, dt.float16, dt.float8e4
    DR = mybir.MatmulPerfMode.DoubleRow

    n_qt = t_len // NQ           # q tiles (4)
    n_tt = t_len // KB           # token tiles of 128 (16)
    n_pr = D // 256              # DoubleRow contraction pairs over D (4)

    nc = bacc.Bacc("TRN2", debug=False, num_devices=NCORES)

    xth = nc.dram_tensor("XH8", [D, t_len], f8, kind="ExternalInput")
    xtl = nc.dram_tensor("XL8", [D, t_len], f8, kind="ExternalInput")
    # weights pre-permuted on host to [128, (a b n)] so the DMA innermost
    # run is 2KB (no small-descriptor penalty)
    wdr = {}
    for wn in ("q", "k", "v"):
        for part in ("h", "l"):
            wdr[wn + part] = nc.dram_tensor(
                f"W{wn.upper()}{part.upper()}8", [128, (D // 128) * GC], f8,
                kind="ExternalInput")
    wo = nc.dram_tensor("WO", [GC, D], f16, kind="ExternalInput")
    ot = nc.dram_tensor("OT", [D, t_len], f16, kind="ExternalOutput")

    with tile.TileContext(nc) as tc:
        with (
            tc.tile_pool(name="w", bufs=1) as wpool,
            tc.tile_pool(name="proj", bufs=1) as proj,
            tc.tile_pool(name="pt0", bufs=16) as ptpool0,
            tc.tile_pool(name="pt1", bufs=18) as ptpool1,
            tc.tile_pool(name="cn", bufs=2) as cnpool,
            tc.tile_pool(name="small", bufs=4) as small,
            tc.tile_pool(name="ot", bufs=8) as otpool,
            tc.tile_pool(name="ps", bufs=2, space="PSUM") as pspool,
            tc.tile_pool(name="st", bufs=2, space="PSUM") as stpool,
            tc.tile_pool(name="cps", bufs=2, space="PSUM") as cpool,
        ):
            # ---- input loads ----
            w_sb = {}

            def load_w(key):
                t_ = wpool.tile([128, n_pr, 2, GC], f8, tag=f"w{key}",
                                name=f"w{key}")
                nc.sync.dma_start(
                    out=t_,
                    in_=wdr[key].ap().rearrange(
                        "p (a b n) -> p a b n", b=2, n=GC),
                )
                w_sb[key] = t_

            # per-n-slice X tiles: DMA-write deps are tile-granular, so a
            # shared tile would stall early projections on later loads
            xh8s = [wpool.tile([128, n_pr, 2, NQ], f8, tag=f"xh8{n}",
                               name=f"xh8{n}") for n in range(n_qt)]
            xl8s = [wpool.tile([128, n_pr, 2, NQ], f8, tag=f"xl8{n}",
                               name=f"xl8{n}") for n in range(n_qt)]

            def load_x(t_, dram, n):
                nc.sync.dma_start(
                    out=t_,
                    in_=dram.ap().rearrange(
                        "(a b p) t -> p a b t", b=2,
                        p=128)[:, :, :, n * NQ:(n + 1) * NQ],
                )

            # n=0 slices + q/k weights first: projections start early
            load_x(xh8s[0], xth, 0)
            load_w("qh")
            load_w("kh")
            load_x(xl8s[0], xtl, 0)
            load_w("ql")
            load_w("kl")
            load_w("vh")
            load_w("vl")
            for n in range(1, n_qt):
                load_x(xh8s[n], xth, n)
                load_x(xl8s[n], xtl, n)
            wo_sb = wpool.tile([128, 2, D], f16, tag="wo")
            nc.sync.dma_start(
                out=wo_sb, in_=wo.ap().rearrange("(c p) n -> p c n", p=128)
            )

            def xslice(xv, c0, c1):
                """[128, pr, 2, c0:c1] view via the per-n-slice tiles."""
                n = c0 // NQ
                assert c1 <= (n + 1) * NQ
                t_ = xh8s[n] if xv == "xh" else xl8s[n]
                return t_[:, :, :, c0 - n * NQ:c1 - n * NQ]

            # ---- persistent sbuf tiles ----
            # Q stored as (hi, lo) fp8 pair in one tile so a single AP
            # spans both DoubleRow k-tiles; K as fp8 hi only (asymmetric S)
            qt8 = [proj.tile([128, 2, t_len], f8, tag=f"qt{m}", name=f"qt{m}")
                   for m in range(2)]
            kt8 = [proj.tile([128, t_len], f8, tag=f"kt{m}", name=f"kt{m}")
                   for m in range(2)]
            v_sb = proj.tile([128, n_tt, HPC, HD + 1], f16, tag="v")
            nc.gpsimd.memset(v_sb[:, :, :, HD:HD + 1], 1.0)
            ct16 = proj.tile([128, 2, t_len], f16, tag="ct16")
            diag_mask = proj.tile([128, 2, KB], f16, tag="dmask")
            nc.gpsimd.memset(diag_mask, 1.0)
            nc.gpsimd.affine_select(
                out=diag_mask,
                in_=diag_mask,
                compare_op=mybir.AluOpType.is_ge,
                fill=0.0,
                base=0,
                pattern=[[0, 2], [1, KB]],
                channel_multiplier=-1,
            )
            ident = proj.tile([128, 128], f16, tag="ident")
            make_identity(nc, ident)

            # ---- projections: 3-term split-precision fp8 DoubleRow ----
            TERMS = (("xh", "h"), ("xl", "h"), ("xh", "l"))

            def proj_qk_unit(wname, dst, hp, n):
                """(X W)^T n-slice for head-pair hp: 12 DR matmuls."""
                ps = pspool.tile([128, NQ], f32, tag="ps", name="ps")
                for t, (xv, wp) in enumerate(TERMS):
                    for qh in range(2):
                        for pr in range(n_pr):
                            nc.tensor.matmul(
                                ps[:, qh * 256:(qh + 1) * 256],
                                w_sb[wname + wp][:, pr, :,
                                                 hp * 128:(hp + 1) * 128],
                                xslice(xv, n * NQ + qh * 256,
                                       n * NQ + (qh + 1) * 256)[:, pr],
                                start=(t == 0 and qh == 0 and pr == 0),
                                stop=(t == 2 and qh == 1 and pr == n_pr - 1),
                                perf_mode=DR,
                            )
                win = slice(n * NQ, (n + 1) * NQ)
                if wname == "q":
                    nc.vector.tensor_copy(qt8[hp][:, 0, win], ps)
                    nc.vector.tensor_sub(
                        qt8[hp][:, 1, win], ps, qt8[hp][:, 0, win])
                else:
                    nc.vector.tensor_copy(kt8[hp][:, win], ps)

            v_done = [0]  # token tiles of V emitted so far (ascending)

            def proj_v_unit(tt):
                assert tt == v_done[0]
                v_done[0] = tt + 1
                ps = pspool.tile([128, NQ], f32, tag="ps", name="ps")
                for t, (xv, wp) in enumerate(TERMS):
                    for pr in range(n_pr):
                        nc.tensor.matmul(
                            ps[:, 0:GC],
                            xslice(xv, tt * 128, (tt + 1) * 128)[:, pr],
                            w_sb["v" + wp][:, pr, :, :],
                            start=(t == 0 and pr == 0),
                            stop=(t == 2 and pr == n_pr - 1),
                            perf_mode=DR,
                        )
                nc.vector.tensor_copy(
                    v_sb[:, tt, :, 0:HD],
                    ps[:, 0:GC].rearrange("p (h d) -> p h d", h=HPC),
                )

            def proj_units(n, qk_only=False, v_only=False):
                units = []
                if not v_only:
                    for hp in range(2):  # q0,k0 first: the S drip needs hp0
                        for wname in ("q", "k"):
                            units.append(
                                lambda w=wname, h=hp:
                                proj_qk_unit(w, None, h, n))
                if not qk_only:
                    for tt in range(4 * n, 4 * n + 4):
                        units.append(lambda t=tt: proj_v_unit(t))
                return units

            filler = []

            def fill(k=1):
                for _ in range(k):
                    if filler:
                        filler.pop(0)()

            # ---- attention pieces ----
            def s_exp(qt, hp, kb, ptpool):
                """fp16 S matmuls (2 heads row-packed) + exp + diag mask."""
                q0 = qt * NQ
                k0 = kb * KB
                off = max(0, k0 - q0)
                st = stpool.tile([128, 2, NQ], f32, tag="st", name="st")
                for i in range(2):
                    qhs = [qh for qh in range(2)
                           if max(off, qh * 256) < (qh + 1) * 256]
                    for qh in qhs:
                        s_off = max(off, qh * 256)
                        qhi = (qh + 1) * 256
                        # DR k-tiles = (Qhi, Qlo) vs duplicated Khi:
                        # S = (Qh+Ql)*Kh at 0.5 cyc/row
                        nc.tensor.matmul(
                            st[:, i, s_off:qhi],
                            kt8[hp][64 * i:64 * i + 64, k0:k0 + KB]
                            .unsqueeze(1).broadcast_to([64, 2, KB]),
                            qt8[hp][64 * i:64 * i + 64, :,
                                    q0 + s_off:q0 + qhi],
                            start=(qh == qhs[0]),
                            stop=(qh == qhs[-1]),
                            perf_mode=DR,
                            tile_position=(64 * i, 0),
                        )
                pt = ptpool.tile([128, 2, NQ], f16, name="pt")
                # S' = 1024*S (Q,K carry 32x) -> scale = 1/(8*1024)
                nc.scalar.activation(
                    out=pt[:, :, off:], in_=st[:, :, off:],
                    func=mybir.ActivationFunctionType.Exp,
                    scale=float(1.0 / (np.sqrt(HD) * 1024.0)),
                )
                if k0 + KB > q0:
                    # only the triangular subtile needs masking; later
                    # subtiles see this key block fully unmasked
                    mw = min(KB, NQ - off)
                    nc.vector.tensor_mul(
                        pt[:, :, off:off + mw],
                        pt[:, :, off:off + mw],
                        diag_mask[:, :, 0:mw],
                    )
                return pt

            def pv_block(qt, hp, kb, pt, c_ps):
                """PV matmuls for one key block: C[q-sub, 65] += PT^T V_aug.

                All four s-subtile groups share one psum bank: single psum
                group — start only on the very first matmul (kb0, s3),
                stop on the last (kb=4qt+3, where only s3 is valid)."""
                off = max(0, kb * KB - qt * NQ)
                for i in range(2):
                    for s in range(3, -1, -1):  # diag (masked) subtile last
                        if off > 128 * s:
                            continue
                        nc.tensor.matmul(
                            c_ps[i][:, s, :],
                            pt[:, i, 128 * s:128 * (s + 1)],
                            v_sb[:, kb, 2 * hp + i, :],
                            start=(kb == 0 and s == 3),
                            stop=(kb == 4 * qt + 3 and s == 3),
                        )

            def epilogue(qt, hp, c_ps, cn):
                """C_norm[q, s, h, d] = C/l into cn (fp16)."""
                for i in range(2):
                    r = small.tile([128, 4, 1], f32, tag="r", name="r")
                    nc.vector.reciprocal(
                        out=r, in_=c_ps[i][:, :, HD:HD + 1])
                    nc.vector.tensor_mul(
                        cn[:, :, 2 * hp + i, :],
                        c_ps[i][:, :, 0:HD],
                        r.broadcast_to([128, 4, HD]),
                    )

            def transp_unit(qt, cn, hp, sp):
                q0 = qt * NQ
                tps = cpool.tile([128, 2, 128], f16, tag="cps", name="tps")
                for u in range(2):  # one psum group for the shared bank
                    nc.tensor.matmul(
                        tps[:, u, :],
                        cn[:, 2 * sp + u, 2 * hp:2 * hp + 2, :],
                        ident,
                        is_transpose=True,
                        start=(u == 0),
                        stop=(u == 1),
                    )
                nc.vector.tensor_copy(
                    ct16[:, hp, q0 + 256 * sp:q0 + 256 * (sp + 1)],
                    tps,
                )

            def out_proj_m(n, m):
                ps = pspool.tile([128, NQ], f32, tag="ps", name="ps")
                for cc in range(2):
                    nc.tensor.matmul(
                        ps,
                        wo_sb[:, cc, m * 128:(m + 1) * 128],
                        ct16[:, cc, n * NQ:(n + 1) * NQ],
                        start=(cc == 0),
                        stop=(cc == 1),
                    )
                o_sb = otpool.tile([128, NQ], f16, name="o_sb")
                nc.vector.tensor_copy(o_sb, ps)
                nc.sync.dma_start(
                    out=ot.ap()[m * 128:(m + 1) * 128,
                                n * NQ:(n + 1) * NQ],
                    in_=o_sb,
                )

            # ---- pipelined schedule ----
            # tail[0]: chunked deferred work from the previous q-tile.
            # cps-tagged chunks (epilogues, wave-B PV, transposes) must all
            # be emitted before this q-tile's c_psA allocation.
            tail = [[]]

            def attention(qt):
                """Unified pipeline: the 2*nkb s_exp stream paces ACT; all
                other PE work is metered between s_exps with dependency-
                aware priority: tail chunks of the previous qt (transposes
                + out-proj; cps ring order requires the transposes before
                this qt's c_psA), then inline PV. Wave-B PV reuses wave-A's
                psum banks right after epilogue-A."""
                nkb = 4 * qt + 4
                inline_b = (qt == n_qt - 1)  # no next qt to hide wave B in
                chunks = tail[0]
                c_psA = [None, None]
                c_psB = [None, None]
                cn = [None]
                phase = ["A"]
                pts = []
                pts1 = []
                pvqA = []
                pvqB = []

                def side_pop():
                    if chunks:
                        chunks.pop(0)()
                        return True
                    if pvqA and v_done[0] > pvqA[0]:
                        kb = pvqA.pop(0)
                        if c_psA[0] is None:
                            c_psA[0] = cpool.tile([128, 4, HD + 1], f32,
                                                  tag="cps", name="cpsA0")
                            c_psA[1] = cpool.tile([128, 4, HD + 1], f32,
                                                  tag="cps", name="cpsA1")
                        pv_block(qt, 0, kb, pts[kb], c_psA)
                        return True
                    if not inline_b:
                        if filler:
                            filler.pop(0)()
                            return True
                        return False
                    if phase[0] == "A" and not pvqA and len(pts) == nkb:
                        cn[0] = cnpool.tile([128, 4, HPC, HD], f16,
                                            tag="cn", name="cn")
                        epilogue(qt, 0, c_psA, cn[0])
                        c_psB[0] = cpool.tile([128, 4, HD + 1], f32,
                                              tag="cps", name="cpsB0")
                        c_psB[1] = cpool.tile([128, 4, HD + 1], f32,
                                              tag="cps", name="cpsB1")
                        phase[0] = "B"
                        return True
                    if (phase[0] == "B" and pvqB
                            and (pvqB[0] <= len(pts1) - 2
                                 or len(pts1) == nkb)):
                        kb = pvqB.pop(0)
                        pv_block(qt, 1, kb, pts1[kb], c_psB)
                        return True
                    if filler:
                        filler.pop(0)()
                        return True
                    return False

                for idx in range(2 * nkb):
                    if idx < nkb:
                        pts.append(s_exp(qt, 0, idx, ptpool0))
                        pvqA.append(idx)
                    else:
                        pts1.append(s_exp(qt, 1, idx - nkb, ptpool1))
                        if inline_b:
                            pvqB.append(idx - nkb)
                    remaining = (len(chunks) + len(pvqA) + len(pvqB)
                                 + len(filler))
                    slots_left = 2 * nkb - idx
                    budget = min(4, -(-remaining // max(1, slots_left)) + 1)
                    for _ in range(budget):
                        if not side_pop():
                            break
                # flush this qt's remaining work (pop fillers to emit any
                # V tiles a pv block still needs)
                while chunks or pvqA or (inline_b and (phase[0] == "A"
                                                       or pvqB)):
                    if not side_pop():
                        if filler:
                            filler.pop(0)()
                        else:
                            break
                if inline_b:
                    epilogue(qt, 1, c_psB, cn[0])
                # next qt's s_exps need its q/k projections emitted
                while len(filler) > 4:
                    filler.pop(0)()

                def make_tail():
                    chunks = []

                    def yield_(n=1):
                        # pad a psum-bank handoff with independent PE work
                        def y():
                            for _ in range(n):
                                if filler:
                                    filler.pop(0)()
                        return y
                    if not inline_b:
                        def epiA():
                            cn[0] = cnpool.tile([128, 4, HPC, HD], f16,
                                                tag="cn", name="cn")
                            epilogue(qt, 0, c_psA, cn[0])
                            c_psB[0] = cpool.tile([128, 4, HD + 1], f32,
                                                  tag="cps", name="cpsB0")
                            c_psB[1] = cpool.tile([128, 4, HD + 1], f32,
                                                  tag="cps", name="cpsB1")
                        chunks.append(epiA)
                        chunks.append(yield_(2))
                        for kb0 in range(0, nkb, 2):
                            def wb(kb0=kb0):
                                for kb in (kb0, kb0 + 1):
                                    pv_block(qt, 1, kb, pts1[kb], c_psB)
                            chunks.append(wb)
                        chunks.append(lambda: epilogue(qt, 1, c_psB, cn[0]))
                        chunks.append(yield_(2))
                    for hp in range(2):
                        for sp in range(2):
                            chunks.append(
                                lambda h=hp, s=sp: transp_unit(qt, cn[0], h, s))
                    for m in range(D // 128):
                        chunks.append(lambda m=m: out_proj_m(qt, m))
                    return chunks

                tail[0] = make_tail()

            for u in proj_units(0, qk_only=True):
                u()
            filler.extend(proj_units(0, v_only=True))
            for qt in range(n_qt):
                if qt + 1 < n_qt:
                    filler.extend(proj_units(qt + 1))
                attention(qt)
            fill(len(filler))
            for ch in tail[0]:
                ch()

    nc.compile()
    return nc


def get_nc(t_len=T):
    if t_len not in _cache:
        _cache[t_len] = _build(t_len)
    return _cache[t_len]


def make_in_maps(X, Wq, Wk, Wv, Wo):
    import ml_dtypes
    f8 = ml_dtypes.float8_e4m3fn
    X = np.asarray(X, dtype=np.float32)
    Wq = np.asarray(Wq, dtype=np.float32)
    Wk = np.asarray(Wk, dtype=np.float32)
    Wv = np.asarray(Wv, dtype=np.float32)
    Wo = np.asarray(Wo, dtype=np.float32)
    in_maps = []
    for c in range(NCORES):
        b, g = divmod(c, 4)
        cols = slice(g * GC, (g + 1) * GC)
        xt = np.ascontiguousarray(X[b].T)
        xh = xt.astype(f8)
        xl = (xt - xh.astype(np.float32)).astype(f8)
        m = {"XH8": xh, "XL8": xl,
             "WO": np.ascontiguousarray(Wo[cols, :] / 32.0)
             .astype(np.float16)}
        for wn, W in (("Q", Wq), ("K", Wk), ("V", Wv)):
            ws = np.ascontiguousarray(W[:, cols]) * 32.0
            wh = ws.astype(f8)
            wl = (ws - wh.astype(np.float32)).astype(f8)
            # permute [d_in, n] -> [128, (a b n)] with d_in = a*256+b*128+p
            def perm(w):
                return np.ascontiguousarray(
                    w.reshape(4, 2, 128, GC).transpose(2, 0, 1, 3)
                    .reshape(128, 8 * GC))
            m[f"W{wn}H8"] = perm(wh)
            m[f"W{wn}L8"] = perm(wl)
        in_maps.append(m)
    return in_maps


def gather_out(results, bo):
    out = np.zeros((B, T, D), dtype=np.float32)
    for c in range(NCORES):
        # device OT = (32*C) @ (Wo/32) = C @ Wo
        out[c // 4] += results[c]["OT"].T.astype(np.float32)
    out += np.asarray(bo, dtype=np.float32)
    return out


def kernel(X, Wq, Wk, Wv, Wo, bo):
    from concourse import bass_utils

    nc = get_nc(T)
    in_maps = make_in_maps(X, Wq, Wk, Wv, Wo)
    res = bass_utils.run_bass_kernel_spmd(
        nc, in_maps, core_ids=list(range(NCORES))
    )
    return gather_out(res.results, bo)
